# revision 39
# baseline (speedup 1.0000x reference)
"""Trainium2 Bass kernel for a 2-layer GraphNetwork (gnn_message_passing).

Strategy (v2):
  - 16 graphs partitioned across 8 cores, 2 graphs per core, paired
    big+small by edge count to balance load. All segment reductions are
    core-local; [16,128] output rows are gathered on the host.
  - Per core, nodes are bin-packed (LPT) into NT tiles of 128 slots; each
    tile's incoming edges are padded to K0 chunks of 128. Segment sums run
    on the tensor engine as one-hot matmuls with HOST-built one-hot tiles.
  - Every hot-loop matmul uses a full K=128 stationary: edge/node feature
    tiles are zero-padded to 128 partitions, with ones/graph-indicator
    rows folded in so biases and global-feature terms are matmul
    accumulations against padded weight tiles. (Partial-K matmuls throttle
    the PE clock to 1.2 GHz; full-K keeps it at 2.4 GHz.)
  - bf16 inputs/intermediates, fp32 PSUM accumulation, fp32 final stage.
"""

import numpy as np
import ml_dtypes

import concourse.bass as bass
import concourse.tile as tile_mod
from concourse import tile
from concourse.bass_utils import run_bass_kernel_spmd
from concourse.vector_clock import ScopedClock

mybir = bass.mybir

N_NODES, N_EDGES, N_GRAPHS = 20000, 320000, 16
F_NODE, F_EDGE, F_GLOB = 64, 32, 16
N_CORES = 8
GPC = N_GRAPHS // N_CORES  # graphs per core = 2

BF16 = mybir.dt.bfloat16
FP8 = mybir.dt.float8e4
F32 = mybir.dt.float32
npbf16 = ml_dtypes.bfloat16
npfp8 = ml_dtypes.float8_e4m3

# ---------------------------------------------------------------------------
# Workaround: CoreV3 codegen rejects instructions carrying more than one
# semaphore wait. Split the waits across extra no-ops.
_MAX_WAITS = 1
_ENGINE_WAIT_LIMIT = 1
_SPLIT_ENGINES = None  # set lazily


def _split_excess_waits(nc):
    global _SPLIT_ENGINES
    if _SPLIT_ENGINES is None:
        ET = mybir.EngineType
        _SPLIT_ENGINES = {ET.PE, ET.Activation, ET.DVE, ET.SP, ET.Pool}
    ctr = [0]
    for bass_bb in nc.bb_map.values():
        bb = bass_bb.bb
        il = bb.instructions
        out = []
        changed = False
        for inst in il:
            si = inst.sync_info
            waits = list(si.on_wait) if (si and si.on_wait) else []
            if len(waits) > _ENGINE_WAIT_LIMIT and inst.engine in _SPLIT_ENGINES:
                head, keep = waits[:-_ENGINE_WAIT_LIMIT], waits[-_ENGINE_WAIT_LIMIT:]
                for i in range(0, len(head), _ENGINE_WAIT_LIMIT):
                    nop = mybir.InstNoOp(name=f"waitsplit-{ctr[0]}", ins=[], outs=[])
                    ctr[0] += 1
                    nop.engine = inst.engine
                    nop.sync_info = mybir.SyncInfo(
                        on_wait=head[i : i + _ENGINE_WAIT_LIMIT], on_update=[]
                    )
                    nc.register_instruction(nop, overwrite=True)
                    out.append(nop)
                inst.sync_info = mybir.SyncInfo(
                    on_wait=keep, on_update=list(si.on_update or [])
                )
                changed = True
            out.append(inst)
        if changed:
            bb.instructions = out


def _split_drain_and_barrier(self, tick_clock, wait_clock):
    nc = self.nc
    _split_excess_waits(nc)
    drain_inst = nc.sync.drain()
    wait_clock.add_sem_waits(
        drain_inst.ins, ScopedClock({None: tick_clock.global_clock})
    )
    mi = drain_inst.ins
    waits = list(mi.sync_info.on_wait) if (mi.sync_info and mi.sync_info.on_wait) else []
    if len(waits) > _MAX_WAITS:
        upd = list(mi.sync_info.on_update) if mi.sync_info.on_update else []
        mi.sync_info = mybir.SyncInfo(on_wait=waits[:_MAX_WAITS], on_update=upd)
        for i in range(_MAX_WAITS, len(waits), _MAX_WAITS):
            nop = nc.sync.nop(nofuse=True)
            nop.ins.sync_info = mybir.SyncInfo(
                on_wait=waits[i : i + _MAX_WAITS], on_update=[]
            )
    nc.all_engine_barrier()
    assert self.sems is not None
    popped = nc._tile_sem_poison_stack.pop()
    assert popped is self._sem_poison
    nc.clear_and_free_semaphores(list(self.sems.allocated().values()))
    nc.all_engine_barrier()


tile_mod.TileContext._drain_and_barrier = _split_drain_and_barrier


# ---------------------------------------------------------------------------
# Host-side graph partitioning / layout


def _pack_core(node_ids, degs, nt, cap_e):
    """LPT: place nodes (descending degree) onto the least-edge-loaded tile
    that still has node capacity. Returns per-tile node-id arrays, or None
    if some tile exceeds cap_e edges."""
    order = np.argsort(-degs, kind="stable")
    tiles_n = [[] for _ in range(nt)]
    tile_ncnt = np.zeros(nt, np.int64)
    tile_ecnt = np.zeros(nt, np.int64)
    for j in order:
        cand = np.where(tile_ncnt < 128)[0]
        if len(cand) == 0:
            return None
        t = cand[np.argmin(tile_ecnt[cand])]
        tiles_n[t].append(node_ids[j])
        tile_ncnt[t] += 1
        tile_ecnt[t] += degs[j]
    if (tile_ecnt > cap_e).any():
        return None
    return [np.array(t, dtype=np.int64) for t in tiles_n]


def _prepare(inputs):
    nf = np.asarray(inputs["node_feats"], np.float32)
    ef = np.asarray(inputs["edge_feats"], np.float32)
    glob = np.asarray(inputs["globals_"], np.float32)
    recv = np.asarray(inputs["receivers"]).astype(np.int64)
    ngraph = np.asarray(inputs["node_graph"]).astype(np.int64)

    cnt = np.bincount(recv, minlength=N_NODES).astype(np.int64)
    egraph = ngraph[recv]
    ncnt_g = np.bincount(ngraph, minlength=N_GRAPHS)
    ecnt_g = np.bincount(egraph, minlength=N_GRAPHS)

    # pair graphs big+small by edge count to balance cores
    order = np.argsort(-ecnt_g, kind="stable")
    pairs = [(int(order[i]), int(order[N_GRAPHS - 1 - i])) for i in range(N_CORES)]

    core_nodes = [
        np.where((ngraph == pa) | (ngraph == pb))[0] for pa, pb in pairs
    ]
    NT = int(max((len(cn) + 127) // 128 for cn in core_nodes))

    core_of_graph = np.zeros(N_GRAPHS, np.int64)
    for c, (pa, pb) in enumerate(pairs):
        core_of_graph[pa] = c
        core_of_graph[pb] = c
    edge_core = core_of_graph[egraph]
    ecnt_core = np.bincount(edge_core, minlength=N_CORES)

    packs = None
    K0 = max(1, int(max(ecnt_core) + NT * 128 - 1) // (NT * 128))
    K0 = ((K0 + 3) // 4) * 4  # quad-chunk device loops need K0 % 4 == 0
    for k0 in range(K0, K0 + 12, 4):
        trial = []
        ok = True
        for c in range(N_CORES):
            p = _pack_core(core_nodes[c], cnt[core_nodes[c]], NT, k0 * 128)
            if p is None:
                ok = False
                break
            trial.append(p)
        if ok:
            packs, K0 = trial, k0
            break
    assert packs is not None, "bin packing failed"

    NPAD = NT * 128
    EPAD = NT * K0 * 128

    # replicated weights, packed: bf16 block and f32 block
    w_np = {}

    def bf(x):
        return np.ascontiguousarray(x).astype(npbf16)

    We1T = np.zeros((128, 256), np.float32)
    We1T[:32] = np.asarray(inputs["We1"], np.float32).T
    We1T[32] = np.asarray(inputs["be1"], np.float32)

    We2T_ = np.asarray(inputs["We2"], np.float32).T  # [256, 128]
    We2T = np.concatenate([We2T_[:128], We2T_[128:]], axis=1)  # [128,256]

    Wn1T = np.zeros((128, 256), np.float32)
    Wn1T[:64] = np.asarray(inputs["Wn1"], np.float32).T
    Wn1T[64] = np.asarray(inputs["bn1"], np.float32)

    Win1T_ = np.asarray(inputs["Win1"], np.float32).T  # [256 fi, 256 fo]
    Win1T = np.concatenate(
        [Win1T_[:128, :128], Win1T_[:128, 128:], Win1T_[128:, :128], Win1T_[128:, 128:]],
        axis=1,
    )  # [128, 512]

    Wn2T_ = np.asarray(inputs["Wn2"], np.float32).T  # [256, 128]
    Wn2T = np.concatenate([Wn2T_[:128], Wn2T_[128:]], axis=1)  # [128,256]
    Win2T = np.asarray(inputs["Win2"], np.float32).T  # [128, 128]
    identb = np.eye(128, dtype=np.float32)

    WggT_pad = np.zeros((128, 128), np.float32)
    WggT_pad[:16] = np.asarray(inputs["Wgg"], np.float32).T
    bgr_pad = np.zeros((128, 128), np.float32)
    bgr_pad[0] = np.asarray(inputs["bg"], np.float32)
    onesc = np.zeros((128, 2), np.float32)
    onesc[0] = 1.0
    w_np["wf"] = np.ascontiguousarray(np.concatenate([
        np.asarray(inputs["Wgn"], np.float32).T,
        np.asarray(inputs["Wge"], np.float32).T,
        WggT_pad, bgr_pad, onesc,
    ], axis=1))  # [128, 514]
    w_np["ident4"] = np.eye(4, dtype=np.float32)

    Wg2 = np.asarray(inputs["Wg2"], np.float32)  # [128, 16]
    Wng2 = np.asarray(inputs["Wng2"], np.float32)
    be2 = np.asarray(inputs["be2"], np.float32)
    bn2 = np.asarray(inputs["bn2"], np.float32)

    # per-core inputs
    in_maps = []
    slot_of_node = np.full(N_NODES, -1, np.int64)
    tile_of_node = np.full(N_NODES, -1, np.int64)
    for c in range(N_CORES):
        pa, pb = pairs[c]
        for t in range(NT):
            ids = packs[c][t]
            slot_of_node[ids] = t * 128 + np.arange(len(ids))
            tile_of_node[ids] = t

        # ---- edges
        eidx = np.where(edge_core == c)[0]
        et = tile_of_node[recv[eidx]]
        order_e = np.argsort(et, kind="stable")
        eidx = eidx[order_e]
        et = et[order_e]
        counts = np.bincount(et, minlength=NT)
        starts = np.concatenate([[0], np.cumsum(counts)[:-1]])
        off_in = np.arange(len(eidx)) - np.repeat(starts, counts)
        dst = et * (K0 * 128) + off_in
        assert (counts <= K0 * 128).all()

        eftT = np.zeros((128, EPAD), np.float32)
        eftT[:32, dst] = ef[eidx].T
        eftT[32, dst] = 1.0
        eg = egraph[eidx]
        eftT[33, dst] = (eg == pa)
        eftT[34, dst] = (eg == pb)

        # one-hot: per 128-edge chunk a [lane, slot] block, K(partitions)=lanes
        onehot = np.zeros((128, EPAD), np.float32)
        onehot[dst % 128, (dst // 128) * 128 + slot_of_node[recv[eidx]] % 128] = 1.0

        # ---- nodes
        slot_node = np.full(NPAD, -1, np.int64)
        for t in range(NT):
            ids = packs[c][t]
            slot_node[t * 128 : t * 128 + len(ids)] = ids
        valid = slot_node >= 0
        sn = np.where(valid, slot_node, 0)

        nftT = np.zeros((128, NPAD), np.float32)
        nftT[:64, valid] = nf[sn[valid]].T
        nftT[64, valid] = 1.0
        ng = ngraph[sn]
        nftT[65] = valid * (ng == pa)
        nftT[66] = valid * (ng == pb)

        invc = np.ones((NPAD, 1), np.float32)
        invc[valid, 0] = 1.0 / np.maximum(cnt[sn[valid]], 1)

        poolw = np.zeros((NPAD, 128), np.float32)
        for g, gid in enumerate((pa, pb)):
            m = valid & (ng == gid)
            poolw[m, g] = 1.0 / max(ncnt_g[gid], 1)
            poolw[m, 2 + g] = cnt[sn[m]] / max(ecnt_g[gid], 1)

        g2aug = np.zeros((128, 128), np.float32)
        g2aug[32] = be2
        g2aug[33] = Wg2 @ glob[pa]
        g2aug[34] = Wg2 @ glob[pb]

        gnaug = np.zeros((128, 128), np.float32)
        gnaug[64] = bn2
        gnaug[65] = Wng2 @ glob[pa]
        gnaug[66] = Wng2 @ glob[pb]

        globT = np.zeros((128, 2), np.float32)
        globT[:16, 0] = glob[pa]
        globT[:16, 1] = glob[pb]

        wb = np.concatenate([We1T, g2aug, We2T, Wn1T, Win1T, Wn2T, Win2T,
                             gnaug, identb], axis=1)  # [128, 2048]

        m = {
            "eft": bf(eftT),
            "onehot": np.ascontiguousarray(onehot).astype(npfp8),
            "nft": bf(nftT),
            "invc": invc,
            "poolw": bf(poolw),
            "wb": bf(wb),
            "globT": globT,
        }
        m.update(w_np)
        in_maps.append(m)

    return in_maps, NT, K0, pairs


# ---------------------------------------------------------------------------
# Device program (identical on all cores)


def _build(NT, K0):
    nc = bass.Bass()
    NPAD = NT * 128
    EPAD = NT * K0 * 128
    CW = K0 * 128  # edge columns per node-tile

    d_eft = nc.dram_tensor("eft", [128, EPAD], BF16, kind="ExternalInput")
    d_onehot = nc.dram_tensor("onehot", [128, EPAD], FP8, kind="ExternalInput")
    d_nft = nc.dram_tensor("nft", [128, NPAD], BF16, kind="ExternalInput")
    d_invc = nc.dram_tensor("invc", [NPAD, 1], F32, kind="ExternalInput")
    d_poolw = nc.dram_tensor("poolw", [NPAD, 128], BF16, kind="ExternalInput")
    d_globT = nc.dram_tensor("globT", [128, 2], F32, kind="ExternalInput")

    d_wb = nc.dram_tensor("wb", [128, 2048], BF16, kind="ExternalInput")
    d_wf = nc.dram_tensor("wf", [128, 514], F32, kind="ExternalInput")
    d_ident4 = nc.dram_tensor("ident4", [4, 4], F32, kind="ExternalInput")

    d_out = nc.dram_tensor("out", [128, 2], F32, kind="ExternalOutput")

    Relu = mybir.ActivationFunctionType.Relu
    Copy = mybir.ActivationFunctionType.Copy
    NQ = K0 // 2   # e1 chunk-pairs per tile
    NR = K0 // 4   # e2 quads per tile
    NS = K0 // 2   # e1T 256-edge slabs per tile

    with tile.TileContext(nc) as tc:
        with tc.tile_pool(name="wp", bufs=1) as wp:
            wbt = wp.tile([128, 2048], BF16, tag="wb")
            nc.gpsimd.dma_start(wbt[:, 0:256], d_wb[:, 0:256])
            nc.gpsimd.dma_start(wbt[:, 256:2048], d_wb[:, 256:2048])
            wft = wp.tile([128, 514], F32, tag="wf")
            nc.gpsimd.dma_start(wft[:], d_wf[:])
            ident4 = wp.tile([4, 4], F32, tag="ident4")
            nc.gpsimd.dma_start(ident4[:], d_ident4[:])
            globT = wp.tile([128, 2], F32, tag="globT")
            nc.gpsimd.dma_start(globT[:], d_globT[:])
            We1T = wbt[:, 0:256]
            g2aug = wbt[:, 256:384]
            We1g = wbt[:, 0:384]
            We2T = wbt[:, 384:640]
            Wn1T = wbt[:, 640:896]
            Win1T = wbt[:, 896:1408]
            Wn2T = wbt[:, 1408:1664]
            Win2T = wbt[:, 1664:1792]
            gnaug = wbt[:, 1792:1920]
            identb = wbt[:, 1920:2048]
            WgnT = wft[:, 0:128]
            WgeT = wft[:, 128:256]
            WggT = wft[:, 256:384]
            bgr = wft[:, 384:512]
            onesc = wft[:, 512:514]

            aggall = wp.tile([128, 384 * NT], BF16, tag="aggall")

            # ----------------- edge phase -----------------
            with tc.tile_pool(name="ep", bufs=3) as ep, \
                 tc.tile_pool(name="e1p", bufs=2) as e1p, \
                 tc.tile_pool(name="efp", bufs=2 * NQ + 2) as efp, \
                 tc.tile_pool(name="psA", bufs=2, space=bass.MemorySpace.PSUM) as psA, \
                 tc.tile_pool(name="psB", bufs=2, space=bass.MemorySpace.PSUM) as psB, \
                 tc.tile_pool(name="psAgg", bufs=2, space=bass.MemorySpace.PSUM) as psAgg:
                for t in range(NT):
                    eftt = ep.tile([128, CW], BF16, tag="eftt", bufs=2)
                    if t == 0:
                        nc.sync.dma_start(eftt[:, 0:512], d_eft[:, 0:512])
                        nc.sync.dma_start(eftt[:, 512:CW], d_eft[:, 512:CW])
                    else:
                        nc.sync.dma_start(eftt[:], d_eft[:, t * CW : (t + 1) * CW])
                    oht = ep.tile([128, CW], FP8, tag="oht", bufs=2)
                    nc.sync.dma_start(oht[:], d_onehot[:, t * CW : (t + 1) * CW])
                    invc_t = ep.tile([128, 1], F32, tag="invc")
                    nc.gpsimd.dma_start(invc_t[:], d_invc[t * 128 : (t + 1) * 128, :])

                    # e1T: feat-major, slab-blocked [h0(256e) | h1(256e)] per slab
                    e1h = e1p.tile([128, 2 * CW], BF16, tag="e1h")
                    for s in range(NS):
                        sl = slice(s * 256, (s + 1) * 256)
                        pT = psA.tile([128, 512], F32, tag="pT")
                        nc.tensor.matmul(pT[:, 0:256], We1T[:, 0:128],
                                         eftt[:, sl], start=True, stop=True)
                        nc.tensor.matmul(pT[:, 256:512], We1T[:, 128:256],
                                         eftt[:, sl], start=True, stop=True)
                        dst = e1h[:, s * 512 : (s + 1) * 512]
                        if s % 2 == 0:
                            nc.scalar.activation(dst, pT[:], Relu)
                        else:
                            nc.vector.tensor_scalar_max(dst, pT[:], 0.0)

                    # fused per-chunk: e1-pre + bias via one N=384 matmul, e2
                    # accumulates onto the bias region; one contiguous relu.
                    efs = []
                    for _q in range(NQ):
                        ef = efp.tile([128, 768], FP8, tag="ef")
                        efs.append(ef)
                    for c in range(K0):
                        csl = slice(c * 128, (c + 1) * 128)
                        pEc = psB.tile([128, 384], F32, tag="pEc", bufs=4)
                        nc.tensor.matmul(pEc[:], eftt[:, csl], We1g,
                                         start=True, stop=False, skip_group_check=True)
                        h0 = (c // 2) * 512 + (c % 2) * 128
                        nc.tensor.matmul(pEc[:, 256:384], e1h[:, h0 : h0 + 128],
                                         We2T[:, 0:128], start=False, stop=False,
                                         skip_group_check=True)
                        nc.tensor.matmul(pEc[:, 256:384], e1h[:, h0 + 256 : h0 + 384],
                                         We2T[:, 128:256], start=False, stop=True,
                                         skip_group_check=True)
                        dst = efs[c // 2][:, (c % 2) * 384 : (c % 2) * 384 + 384]
                        if c % 2 == 0 and c not in (6, 14):
                            nc.scalar.activation(dst, pEc[:], Relu)
                        else:
                            nc.vector.tensor_scalar_max(dst, pEc[:], 0.0)

                    # aggregation: fp8 DoubleRow, 256 edges (2 chunks) per matmul
                    pagg = psAgg.tile([128, 384], F32, tag="pagg")
                    for q in range(NQ):
                        lhs3 = oht[:, q * 256 : (q + 1) * 256].rearrange(
                            "k (p m) -> k p m", p=2)
                        rhs3 = efs[q][:].rearrange("k (p n) -> k p n", p=2)
                        nc.tensor.matmul(pagg[:], lhs3, rhs3,
                                         start=(q == 0), stop=(q == NQ - 1),
                                         perf_mode=mybir.MatmulPerfMode.DoubleRow)

                    nc.scalar.activation(
                        aggall[:, t * 384 : (t + 1) * 384], pagg[:], Copy,
                        scale=invc_t[:],
                    )

            # ----------------- node phase -----------------
            with tc.tile_pool(name="np_", bufs=NT) as np_, \
                 tc.tile_pool(name="agp", bufs=NT) as agp, \
                 tc.tile_pool(name="nsb", bufs=3) as nsb, \
                 tc.tile_pool(name="npsT", bufs=2, space=bass.MemorySpace.PSUM) as npsT, \
                 tc.tile_pool(name="npsB", bufs=2, space=bass.MemorySpace.PSUM) as npsB, \
                 tc.tile_pool(name="npsC", bufs=2, space=bass.MemorySpace.PSUM) as npsC, \
                 tc.tile_pool(name="npsP", bufs=1, space=bass.MemorySpace.PSUM) as npsP:
                ppNE = npsP.tile([128, 256], F32, tag="ppNE")

                nftts, pws = [], []
                for t in range(NT):
                    nftt = np_.tile([128, 128], BF16, tag="nftt")
                    nc.gpsimd.dma_start(nftt[:], d_nft[:, t * 128 : (t + 1) * 128])
                    pw = np_.tile([128, 128], BF16, tag="pw")
                    nc.gpsimd.dma_start(pw[:], d_poolw[t * 128 : (t + 1) * 128, :])
                    nftts.append(nftt)
                    pws.append(pw)

                # pass 1: transpose agg tiles to feat-major
                aggTs = []
                for t in range(NT):
                    aggsl = aggall[:, t * 384 : (t + 1) * 384]
                    pT = npsT.tile([128, 384], BF16, tag="pT")
                    nc.tensor.transpose(pT[:, 0:128], aggsl[:, 0:128], identb)
                    nc.tensor.transpose(pT[:, 128:256], aggsl[:, 128:256], identb)
                    nc.tensor.transpose(pT[:, 256:384], aggsl[:, 256:384], identb)
                    aggT = agp.tile([128, 384], BF16, tag="aggT")
                    nc.vector.tensor_copy(aggT[:], pT[:])
                    aggTs.append(aggT)

                # pass 2: node MLPs + pooling (pn1 emitted one tile ahead)
                def emit_pn1(t):
                    nftt = nftts[t]
                    aggT = aggTs[t]
                    pn1 = npsB.tile([128, 256], F32, tag="pn1")
                    for s in (0, 1):
                        ssl = slice(s * 128, (s + 1) * 128)
                        nc.tensor.matmul(pn1[:, ssl], Wn1T[:, ssl], nftt[:],
                                         start=True, stop=False)
                        nc.tensor.matmul(pn1[:, ssl], Win1T[:, s * 128 : s * 128 + 128],
                                         aggT[:, 0:128], start=False, stop=False)
                        nc.tensor.matmul(pn1[:, ssl], Win1T[:, 256 + s * 128 : 256 + s * 128 + 128],
                                         aggT[:, 128:256], start=False, stop=True)
                    return pn1

                pn1_cur = emit_pn1(0)
                for t in range(NT):
                    aggsl = aggall[:, t * 384 : (t + 1) * 384]
                    aggT = aggTs[t]
                    nftt = nftts[t]
                    pw = pws[t]

                    n1T = nsb.tile([128, 256], BF16, tag="n1T")
                    nc.scalar.activation(n1T[:], pn1_cur[:], Relu)
                    if t + 1 < NT:
                        pn1_cur = emit_pn1(t + 1)

                    pn2 = npsC.tile([128, 128], F32, tag="pn2")
                    nc.tensor.matmul(pn2[:], n1T[:, 0:128], Wn2T[:, 0:128], start=True, stop=False)
                    nc.tensor.matmul(pn2[:], n1T[:, 128:256], Wn2T[:, 128:256], start=False, stop=False)
                    nc.tensor.matmul(pn2[:], aggT[:, 256:384], Win2T, start=False, stop=False)
                    nc.tensor.matmul(pn2[:], nftt[:], gnaug, start=False, stop=True)
                    n2 = nsb.tile([128, 128], BF16, tag="n2")
                    nc.vector.tensor_scalar_max(n2[:], pn2[:], 0.0)

                    nc.tensor.matmul(ppNE[:, 0:128], pw[:], n2[:],
                                     start=(t == 0), stop=(t == NT - 1))

                # edge-average pooling: separate sequential group (same bank)
                for t in range(NT):
                    nc.tensor.matmul(ppNE[:, 128:256], pws[t][:],
                                     aggall[:, t * 384 + 256 : (t + 1) * 384],
                                     start=(t == 0), stop=(t == NT - 1))

                # ----------------- final projection -----------------
                pp4 = nsb.tile([4, 256], F32, tag="pp4")
                nc.scalar.activation(pp4[:], ppNE[0:4, :], Copy)

                ptail = npsP.tile([128, 16], F32, tag="ptail")
                nc.tensor.transpose(ptail[:, 0:4], pp4[:, 0:128], ident4[:])
                nc.tensor.transpose(ptail[:, 4:8], pp4[:, 128:256], ident4[:])
                nt8 = nsb.tile([128, 8], F32, tag="nt8")
                nc.scalar.activation(nt8[:], ptail[:, 0:8], Copy)

                nc.tensor.matmul(ptail[:, 8:10], WgnT, nt8[:, 0:2], start=True, stop=False)
                nc.tensor.matmul(ptail[:, 8:10], WgeT, nt8[:, 6:8], start=False, stop=False)
                nc.tensor.matmul(ptail[:, 8:10], WggT, globT[:], start=False, stop=False)
                nc.tensor.matmul(ptail[:, 8:10], bgr, onesc, start=False, stop=True)
                outsb = nsb.tile([128, 2], F32, tag="outsb")
                nc.scalar.activation(outsb[:], ptail[:, 8:10], Copy)
                nc.sync.dma_start(d_out[:], outsb[:])

    return nc


_CACHE = {}


def _get_nc(NT, K0):
    key = (NT, K0)
    if key not in _CACHE:
        _CACHE[key] = _build(NT, K0)
    return _CACHE[key]


def _run(inputs, trace=False):
    in_maps, NT, K0, pairs = _prepare(inputs)
    nc = _get_nc(NT, K0)
    res = run_bass_kernel_spmd(nc, in_maps, list(range(N_CORES)), trace=trace)
    out = np.zeros((N_GRAPHS, 128), np.float32)
    for c in range(N_CORES):
        r = np.asarray(res.results[c]["out"], np.float32)
        pa, pb = pairs[c]
        out[pa] = r[:, 0]
        out[pb] = r[:, 1]
    return out, res


def kernel(**inputs):
    out, _ = _run(inputs, trace=False)
    return out


def kernel_traced(**inputs):
    return _run(inputs, trace=True)


# revision 40
# speedup vs baseline: 1.0551x; 1.0551x over previous
"""Trainium2 Bass kernel for a 2-layer GraphNetwork (gnn_message_passing).

Strategy (v2):
  - 16 graphs partitioned across 8 cores, 2 graphs per core, paired
    big+small by edge count to balance load. All segment reductions are
    core-local; [16,128] output rows are gathered on the host.
  - Per core, nodes are bin-packed (LPT) into NT tiles of 128 slots; each
    tile's incoming edges are padded to K0 chunks of 128. Segment sums run
    on the tensor engine as one-hot matmuls with HOST-built one-hot tiles.
  - Every hot-loop matmul uses a full K=128 stationary: edge/node feature
    tiles are zero-padded to 128 partitions, with ones/graph-indicator
    rows folded in so biases and global-feature terms are matmul
    accumulations against padded weight tiles. (Partial-K matmuls throttle
    the PE clock to 1.2 GHz; full-K keeps it at 2.4 GHz.)
  - bf16 inputs/intermediates, fp32 PSUM accumulation, fp32 final stage.
"""

import numpy as np
import ml_dtypes

import concourse.bass as bass
import concourse.tile as tile_mod
from concourse import tile
from concourse.bass_utils import run_bass_kernel_spmd
from concourse.vector_clock import ScopedClock

mybir = bass.mybir

N_NODES, N_EDGES, N_GRAPHS = 20000, 320000, 16
F_NODE, F_EDGE, F_GLOB = 64, 32, 16
N_CORES = 8
GPC = N_GRAPHS // N_CORES  # graphs per core = 2

BF16 = mybir.dt.bfloat16
FP8 = mybir.dt.float8e4
F32 = mybir.dt.float32
npbf16 = ml_dtypes.bfloat16
npfp8 = ml_dtypes.float8_e4m3

# ---------------------------------------------------------------------------
# Workaround: CoreV3 codegen rejects instructions carrying more than one
# semaphore wait. Split the waits across extra no-ops.
_MAX_WAITS = 1
_ENGINE_WAIT_LIMIT = 1
_SPLIT_ENGINES = None  # set lazily


def _split_excess_waits(nc):
    global _SPLIT_ENGINES
    if _SPLIT_ENGINES is None:
        ET = mybir.EngineType
        _SPLIT_ENGINES = {ET.PE, ET.Activation, ET.DVE, ET.SP, ET.Pool}
    ctr = [0]
    for bass_bb in nc.bb_map.values():
        bb = bass_bb.bb
        il = bb.instructions
        out = []
        changed = False
        for inst in il:
            si = inst.sync_info
            waits = list(si.on_wait) if (si and si.on_wait) else []
            if len(waits) > _ENGINE_WAIT_LIMIT and inst.engine in _SPLIT_ENGINES:
                head, keep = waits[:-_ENGINE_WAIT_LIMIT], waits[-_ENGINE_WAIT_LIMIT:]
                for i in range(0, len(head), _ENGINE_WAIT_LIMIT):
                    nop = mybir.InstNoOp(name=f"waitsplit-{ctr[0]}", ins=[], outs=[])
                    ctr[0] += 1
                    nop.engine = inst.engine
                    nop.sync_info = mybir.SyncInfo(
                        on_wait=head[i : i + _ENGINE_WAIT_LIMIT], on_update=[]
                    )
                    nc.register_instruction(nop, overwrite=True)
                    out.append(nop)
                inst.sync_info = mybir.SyncInfo(
                    on_wait=keep, on_update=list(si.on_update or [])
                )
                changed = True
            out.append(inst)
        if changed:
            bb.instructions = out


def _split_drain_and_barrier(self, tick_clock, wait_clock):
    nc = self.nc
    _split_excess_waits(nc)
    drain_inst = nc.sync.drain()
    wait_clock.add_sem_waits(
        drain_inst.ins, ScopedClock({None: tick_clock.global_clock})
    )
    mi = drain_inst.ins
    waits = list(mi.sync_info.on_wait) if (mi.sync_info and mi.sync_info.on_wait) else []
    if len(waits) > _MAX_WAITS:
        upd = list(mi.sync_info.on_update) if mi.sync_info.on_update else []
        mi.sync_info = mybir.SyncInfo(on_wait=waits[:_MAX_WAITS], on_update=upd)
        for i in range(_MAX_WAITS, len(waits), _MAX_WAITS):
            nop = nc.sync.nop(nofuse=True)
            nop.ins.sync_info = mybir.SyncInfo(
                on_wait=waits[i : i + _MAX_WAITS], on_update=[]
            )
    nc.all_engine_barrier()
    assert self.sems is not None
    popped = nc._tile_sem_poison_stack.pop()
    assert popped is self._sem_poison
    nc.clear_and_free_semaphores(list(self.sems.allocated().values()))
    nc.all_engine_barrier()


tile_mod.TileContext._drain_and_barrier = _split_drain_and_barrier


# ---------------------------------------------------------------------------
# Host-side graph partitioning / layout


def _pack_core(node_ids, degs, nt, cap_e):
    """LPT: place nodes (descending degree) onto the least-edge-loaded tile
    that still has node capacity. Returns per-tile node-id arrays, or None
    if some tile exceeds cap_e edges."""
    order = np.argsort(-degs, kind="stable")
    tiles_n = [[] for _ in range(nt)]
    tile_ncnt = np.zeros(nt, np.int64)
    tile_ecnt = np.zeros(nt, np.int64)
    for j in order:
        cand = np.where(tile_ncnt < 128)[0]
        if len(cand) == 0:
            return None
        t = cand[np.argmin(tile_ecnt[cand])]
        tiles_n[t].append(node_ids[j])
        tile_ncnt[t] += 1
        tile_ecnt[t] += degs[j]
    if (tile_ecnt > cap_e).any():
        return None
    return [np.array(t, dtype=np.int64) for t in tiles_n]


def _prepare(inputs):
    nf = np.asarray(inputs["node_feats"], np.float32)
    ef = np.asarray(inputs["edge_feats"], np.float32)
    glob = np.asarray(inputs["globals_"], np.float32)
    recv = np.asarray(inputs["receivers"]).astype(np.int64)
    ngraph = np.asarray(inputs["node_graph"]).astype(np.int64)

    cnt = np.bincount(recv, minlength=N_NODES).astype(np.int64)
    egraph = ngraph[recv]
    ncnt_g = np.bincount(ngraph, minlength=N_GRAPHS)
    ecnt_g = np.bincount(egraph, minlength=N_GRAPHS)

    # pair graphs big+small by edge count to balance cores
    order = np.argsort(-ecnt_g, kind="stable")
    pairs = [(int(order[i]), int(order[N_GRAPHS - 1 - i])) for i in range(N_CORES)]

    core_nodes = [
        np.where((ngraph == pa) | (ngraph == pb))[0] for pa, pb in pairs
    ]
    NT = int(max((len(cn) + 127) // 128 for cn in core_nodes))

    core_of_graph = np.zeros(N_GRAPHS, np.int64)
    for c, (pa, pb) in enumerate(pairs):
        core_of_graph[pa] = c
        core_of_graph[pb] = c
    edge_core = core_of_graph[egraph]
    ecnt_core = np.bincount(edge_core, minlength=N_CORES)

    packs = None
    K0 = max(1, int(max(ecnt_core) + NT * 128 - 1) // (NT * 128))
    K0 = ((K0 + 3) // 4) * 4  # quad-chunk device loops need K0 % 4 == 0
    for k0 in range(K0, K0 + 12, 4):
        trial = []
        ok = True
        for c in range(N_CORES):
            p = _pack_core(core_nodes[c], cnt[core_nodes[c]], NT, k0 * 128)
            if p is None:
                ok = False
                break
            trial.append(p)
        if ok:
            packs, K0 = trial, k0
            break
    assert packs is not None, "bin packing failed"

    NPAD = NT * 128
    EPAD = NT * K0 * 128

    # replicated weights, packed: bf16 block and f32 block
    w_np = {}

    def bf(x):
        return np.ascontiguousarray(x).astype(npbf16)

    We1T = np.zeros((128, 256), np.float32)
    We1T[:32] = np.asarray(inputs["We1"], np.float32).T
    We1T[32] = np.asarray(inputs["be1"], np.float32)

    We2T_ = np.asarray(inputs["We2"], np.float32).T  # [256, 128]
    We2T = np.concatenate([We2T_[:128], We2T_[128:]], axis=1)  # [128,256]

    Wn1T = np.zeros((128, 256), np.float32)
    Wn1T[:64] = np.asarray(inputs["Wn1"], np.float32).T
    Wn1T[64] = np.asarray(inputs["bn1"], np.float32)

    Win1T_ = np.asarray(inputs["Win1"], np.float32).T  # [256 fi, 256 fo]
    Win1T = np.concatenate(
        [Win1T_[:128, :128], Win1T_[:128, 128:], Win1T_[128:, :128], Win1T_[128:, 128:]],
        axis=1,
    )  # [128, 512]

    Wn2T_ = np.asarray(inputs["Wn2"], np.float32).T  # [256, 128]
    Wn2T = np.concatenate([Wn2T_[:128], Wn2T_[128:]], axis=1)  # [128,256]
    Win2T = np.asarray(inputs["Win2"], np.float32).T  # [128, 128]
    identb = np.eye(128, dtype=np.float32)

    WggT_pad = np.zeros((128, 128), np.float32)
    WggT_pad[:16] = np.asarray(inputs["Wgg"], np.float32).T
    bgr_pad = np.zeros((128, 128), np.float32)
    bgr_pad[0] = np.asarray(inputs["bg"], np.float32)
    onesc = np.zeros((128, 2), np.float32)
    onesc[0] = 1.0
    w_np["wf"] = np.ascontiguousarray(np.concatenate([
        np.asarray(inputs["Wgn"], np.float32).T,
        np.asarray(inputs["Wge"], np.float32).T,
        WggT_pad, bgr_pad, onesc,
    ], axis=1))  # [128, 514]
    w_np["ident4"] = np.eye(4, dtype=np.float32)

    Wg2 = np.asarray(inputs["Wg2"], np.float32)  # [128, 16]
    Wng2 = np.asarray(inputs["Wng2"], np.float32)
    be2 = np.asarray(inputs["be2"], np.float32)
    bn2 = np.asarray(inputs["bn2"], np.float32)

    # per-core inputs
    in_maps = []
    slot_of_node = np.full(N_NODES, -1, np.int64)
    tile_of_node = np.full(N_NODES, -1, np.int64)
    for c in range(N_CORES):
        pa, pb = pairs[c]
        for t in range(NT):
            ids = packs[c][t]
            slot_of_node[ids] = t * 128 + np.arange(len(ids))
            tile_of_node[ids] = t

        # ---- edges
        eidx = np.where(edge_core == c)[0]
        et = tile_of_node[recv[eidx]]
        order_e = np.argsort(et, kind="stable")
        eidx = eidx[order_e]
        et = et[order_e]
        counts = np.bincount(et, minlength=NT)
        starts = np.concatenate([[0], np.cumsum(counts)[:-1]])
        off_in = np.arange(len(eidx)) - np.repeat(starts, counts)
        dst = et * (K0 * 128) + off_in
        assert (counts <= K0 * 128).all()

        eftT = np.zeros((128, EPAD), np.float32)
        eftT[:32, dst] = ef[eidx].T
        eftT[32, dst] = 1.0
        eg = egraph[eidx]
        eftT[33, dst] = (eg == pa)
        eftT[34, dst] = (eg == pb)

        # one-hot: per 128-edge chunk a [lane, slot] block, K(partitions)=lanes
        onehot = np.zeros((128, EPAD), np.float32)
        onehot[dst % 128, (dst // 128) * 128 + slot_of_node[recv[eidx]] % 128] = 1.0

        # ---- nodes
        slot_node = np.full(NPAD, -1, np.int64)
        for t in range(NT):
            ids = packs[c][t]
            slot_node[t * 128 : t * 128 + len(ids)] = ids
        valid = slot_node >= 0
        sn = np.where(valid, slot_node, 0)

        nftT = np.zeros((128, NPAD), np.float32)
        nftT[:64, valid] = nf[sn[valid]].T
        nftT[64, valid] = 1.0
        ng = ngraph[sn]
        nftT[65] = valid * (ng == pa)
        nftT[66] = valid * (ng == pb)

        invc = np.ones((NPAD, 1), np.float32)
        invc[valid, 0] = 1.0 / np.maximum(cnt[sn[valid]], 1)

        poolw = np.zeros((NPAD, 128), np.float32)
        for g, gid in enumerate((pa, pb)):
            m = valid & (ng == gid)
            poolw[m, g] = 1.0 / max(ncnt_g[gid], 1)
            poolw[m, 2 + g] = cnt[sn[m]] / max(ecnt_g[gid], 1)

        g2aug = np.zeros((128, 128), np.float32)
        g2aug[32] = be2
        g2aug[33] = Wg2 @ glob[pa]
        g2aug[34] = Wg2 @ glob[pb]

        gnaug = np.zeros((128, 128), np.float32)
        gnaug[64] = bn2
        gnaug[65] = Wng2 @ glob[pa]
        gnaug[66] = Wng2 @ glob[pb]

        globT = np.zeros((128, 2), np.float32)
        globT[:16, 0] = glob[pa]
        globT[:16, 1] = glob[pb]

        wb = np.concatenate([We1T, g2aug, We2T, Wn1T, Win1T, Wn2T, Win2T,
                             gnaug, identb], axis=1)  # [128, 2048]

        m = {
            "eft": bf(eftT),
            "onehot": np.ascontiguousarray(onehot).astype(npfp8),
            "nft": bf(nftT),
            "invc": invc,
            "poolw": bf(poolw),
            "wb": bf(wb),
            "globT": globT,
        }
        m.update(w_np)
        in_maps.append(m)

    return in_maps, NT, K0, pairs


# ---------------------------------------------------------------------------
# Device program (identical on all cores)


def _build(NT, K0):
    nc = bass.Bass()
    NPAD = NT * 128
    EPAD = NT * K0 * 128
    CW = K0 * 128  # edge columns per node-tile

    d_eft = nc.dram_tensor("eft", [128, EPAD], BF16, kind="ExternalInput")
    d_onehot = nc.dram_tensor("onehot", [128, EPAD], FP8, kind="ExternalInput")
    d_nft = nc.dram_tensor("nft", [128, NPAD], BF16, kind="ExternalInput")
    d_invc = nc.dram_tensor("invc", [NPAD, 1], F32, kind="ExternalInput")
    d_poolw = nc.dram_tensor("poolw", [NPAD, 128], BF16, kind="ExternalInput")
    d_globT = nc.dram_tensor("globT", [128, 2], F32, kind="ExternalInput")

    d_wb = nc.dram_tensor("wb", [128, 2048], BF16, kind="ExternalInput")
    d_wf = nc.dram_tensor("wf", [128, 514], F32, kind="ExternalInput")
    d_ident4 = nc.dram_tensor("ident4", [4, 4], F32, kind="ExternalInput")

    d_out = nc.dram_tensor("out", [128, 2], F32, kind="ExternalOutput")

    Relu = mybir.ActivationFunctionType.Relu
    Copy = mybir.ActivationFunctionType.Copy
    NQ = K0 // 2   # e1 chunk-pairs per tile
    NR = K0 // 4   # e2 quads per tile
    NS = K0 // 2   # e1T 256-edge slabs per tile

    with tile.TileContext(nc) as tc:
        with tc.tile_pool(name="wp", bufs=1) as wp:
            wbt = wp.tile([128, 2048], BF16, tag="wb")
            nc.gpsimd.dma_start(wbt[:, 0:256], d_wb[:, 0:256])
            nc.gpsimd.dma_start(wbt[:, 256:2048], d_wb[:, 256:2048])
            wft = wp.tile([128, 514], F32, tag="wf")
            nc.gpsimd.dma_start(wft[:], d_wf[:])
            ident4 = wp.tile([4, 4], F32, tag="ident4")
            nc.gpsimd.dma_start(ident4[:], d_ident4[:])
            globT = wp.tile([128, 2], F32, tag="globT")
            nc.gpsimd.dma_start(globT[:], d_globT[:])
            We1T = wbt[:, 0:256]
            g2aug = wbt[:, 256:384]
            We1g = wbt[:, 0:384]
            We2T = wbt[:, 384:640]
            Wn1T = wbt[:, 640:896]
            Win1T = wbt[:, 896:1408]
            Wn2T = wbt[:, 1408:1664]
            Win2T = wbt[:, 1664:1792]
            gnaug = wbt[:, 1792:1920]
            identb = wbt[:, 1920:2048]
            WgnT = wft[:, 0:128]
            WgeT = wft[:, 128:256]
            WggT = wft[:, 256:384]
            bgr = wft[:, 384:512]
            onesc = wft[:, 512:514]

            aggall = wp.tile([128, 384 * NT], BF16, tag="aggall")

            # ----------------- edge phase -----------------
            with tc.tile_pool(name="ep", bufs=3) as ep, \
                 tc.tile_pool(name="e1p", bufs=2) as e1p, \
                 tc.tile_pool(name="efp", bufs=2 * NQ + 2) as efp, \
                 tc.tile_pool(name="psA", bufs=2, space=bass.MemorySpace.PSUM) as psA, \
                 tc.tile_pool(name="psB", bufs=2, space=bass.MemorySpace.PSUM) as psB, \
                 tc.tile_pool(name="psAgg", bufs=2, space=bass.MemorySpace.PSUM) as psAgg:
                for t in range(NT):
                    eftt = ep.tile([128, CW], BF16, tag="eftt", bufs=2)
                    if t == 0:
                        nc.sync.dma_start(eftt[:, 0:512], d_eft[:, 0:512])
                        nc.sync.dma_start(eftt[:, 512:CW], d_eft[:, 512:CW])
                    else:
                        nc.sync.dma_start(eftt[:], d_eft[:, t * CW : (t + 1) * CW])
                    oht = ep.tile([128, CW], FP8, tag="oht", bufs=2)
                    nc.sync.dma_start(oht[:], d_onehot[:, t * CW : (t + 1) * CW])
                    invc_t = ep.tile([128, 1], F32, tag="invc")
                    nc.gpsimd.dma_start(invc_t[:], d_invc[t * 128 : (t + 1) * 128, :])

                    # e1T: feat-major, slab-blocked [h0(256e) | h1(256e)] per slab
                    e1h = e1p.tile([128, 2 * CW], BF16, tag="e1h")
                    for s in range(NS):
                        sl = slice(s * 256, (s + 1) * 256)
                        pT = psA.tile([128, 512], F32, tag="pT")
                        nc.tensor.matmul(pT[:, 0:256], We1T[:, 0:128],
                                         eftt[:, sl], start=True, stop=True)
                        nc.tensor.matmul(pT[:, 256:512], We1T[:, 128:256],
                                         eftt[:, sl], start=True, stop=True)
                        dst = e1h[:, s * 512 : (s + 1) * 512]
                        if s % 2 == 0:
                            nc.scalar.activation(dst, pT[:], Relu)
                        else:
                            nc.vector.tensor_scalar_max(dst, pT[:], 0.0)

                    # fused per-chunk: e1-pre + bias via one N=384 matmul, e2
                    # accumulates onto the bias region; one contiguous relu.
                    efs = []
                    for _q in range(NQ):
                        ef = efp.tile([128, 768], FP8, tag="ef")
                        efs.append(ef)
                    for c in range(K0):
                        csl = slice(c * 128, (c + 1) * 128)
                        pEc = psB.tile([128, 384], F32, tag="pEc", bufs=4)
                        nc.tensor.matmul(pEc[:], eftt[:, csl], We1g,
                                         start=True, stop=False, skip_group_check=True)
                        h0 = (c // 2) * 512 + (c % 2) * 128
                        nc.tensor.matmul(pEc[:, 256:384], e1h[:, h0 : h0 + 128],
                                         We2T[:, 0:128], start=False, stop=False,
                                         skip_group_check=True)
                        nc.tensor.matmul(pEc[:, 256:384], e1h[:, h0 + 256 : h0 + 384],
                                         We2T[:, 128:256], start=False, stop=True,
                                         skip_group_check=True)
                        dst = efs[c // 2][:, (c % 2) * 384 : (c % 2) * 384 + 384]
                        if c % 2 == 0:
                            nc.scalar.activation(dst, pEc[:], Relu)
                        else:
                            nc.vector.tensor_scalar_max(dst, pEc[:], 0.0)

                    # aggregation: fp8 DoubleRow, 256 edges (2 chunks) per matmul
                    pagg = psAgg.tile([128, 384], F32, tag="pagg")
                    for q in range(NQ):
                        lhs3 = oht[:, q * 256 : (q + 1) * 256].rearrange(
                            "k (p m) -> k p m", p=2)
                        rhs3 = efs[q][:].rearrange("k (p n) -> k p n", p=2)
                        nc.tensor.matmul(pagg[:], lhs3, rhs3,
                                         start=(q == 0), stop=(q == NQ - 1),
                                         perf_mode=mybir.MatmulPerfMode.DoubleRow)

                    nc.scalar.activation(
                        aggall[:, t * 384 : (t + 1) * 384], pagg[:], Copy,
                        scale=invc_t[:],
                    )

            # ----------------- node phase -----------------
            with tc.tile_pool(name="np_", bufs=NT) as np_, \
                 tc.tile_pool(name="agp", bufs=NT) as agp, \
                 tc.tile_pool(name="nsb", bufs=3) as nsb, \
                 tc.tile_pool(name="npsT", bufs=2, space=bass.MemorySpace.PSUM) as npsT, \
                 tc.tile_pool(name="npsB", bufs=2, space=bass.MemorySpace.PSUM) as npsB, \
                 tc.tile_pool(name="npsC", bufs=2, space=bass.MemorySpace.PSUM) as npsC, \
                 tc.tile_pool(name="npsP", bufs=1, space=bass.MemorySpace.PSUM) as npsP:
                ppNE = npsP.tile([128, 256], F32, tag="ppNE")

                nftts, pws = [], []
                for t in range(NT):
                    nftt = np_.tile([128, 128], BF16, tag="nftt")
                    nc.gpsimd.dma_start(nftt[:], d_nft[:, t * 128 : (t + 1) * 128])
                    pw = np_.tile([128, 128], BF16, tag="pw")
                    nc.gpsimd.dma_start(pw[:], d_poolw[t * 128 : (t + 1) * 128, :])
                    nftts.append(nftt)
                    pws.append(pw)

                # pass 1: transpose agg tiles to feat-major
                aggTs = []
                for t in range(NT):
                    aggsl = aggall[:, t * 384 : (t + 1) * 384]
                    pT = npsT.tile([128, 384], BF16, tag="pT")
                    nc.tensor.transpose(pT[:, 0:128], aggsl[:, 0:128], identb)
                    nc.tensor.transpose(pT[:, 128:256], aggsl[:, 128:256], identb)
                    nc.tensor.transpose(pT[:, 256:384], aggsl[:, 256:384], identb)
                    aggT = agp.tile([128, 384], BF16, tag="aggT")
                    nc.vector.tensor_copy(aggT[:], pT[:])
                    aggTs.append(aggT)

                # pass 2: node MLPs + pooling (pn1 emitted one tile ahead)
                def emit_pn1(t):
                    nftt = nftts[t]
                    aggT = aggTs[t]
                    pn1 = npsB.tile([128, 256], F32, tag="pn1")
                    for s in (0, 1):
                        ssl = slice(s * 128, (s + 1) * 128)
                        nc.tensor.matmul(pn1[:, ssl], Wn1T[:, ssl], nftt[:],
                                         start=True, stop=False)
                        nc.tensor.matmul(pn1[:, ssl], Win1T[:, s * 128 : s * 128 + 128],
                                         aggT[:, 0:128], start=False, stop=False)
                        nc.tensor.matmul(pn1[:, ssl], Win1T[:, 256 + s * 128 : 256 + s * 128 + 128],
                                         aggT[:, 128:256], start=False, stop=True)
                    return pn1

                pn1_cur = emit_pn1(0)
                for t in range(NT):
                    aggsl = aggall[:, t * 384 : (t + 1) * 384]
                    aggT = aggTs[t]
                    nftt = nftts[t]
                    pw = pws[t]

                    n1T = nsb.tile([128, 256], BF16, tag="n1T")
                    nc.scalar.activation(n1T[:], pn1_cur[:], Relu)
                    if t + 1 < NT:
                        pn1_cur = emit_pn1(t + 1)

                    pn2 = npsC.tile([128, 128], F32, tag="pn2")
                    nc.tensor.matmul(pn2[:], n1T[:, 0:128], Wn2T[:, 0:128], start=True, stop=False)
                    nc.tensor.matmul(pn2[:], n1T[:, 128:256], Wn2T[:, 128:256], start=False, stop=False)
                    nc.tensor.matmul(pn2[:], aggT[:, 256:384], Win2T, start=False, stop=False)
                    nc.tensor.matmul(pn2[:], nftt[:], gnaug, start=False, stop=True)
                    n2 = nsb.tile([128, 128], BF16, tag="n2")
                    nc.vector.tensor_scalar_max(n2[:], pn2[:], 0.0)

                    nc.tensor.matmul(ppNE[:, 0:128], pw[:], n2[:],
                                     start=(t == 0), stop=(t == NT - 1))

                # edge-average pooling: separate sequential group (same bank)
                for t in range(NT):
                    nc.tensor.matmul(ppNE[:, 128:256], pws[t][:],
                                     aggall[:, t * 384 + 256 : (t + 1) * 384],
                                     start=(t == 0), stop=(t == NT - 1))

                # ----------------- final projection -----------------
                pp4 = nsb.tile([4, 256], F32, tag="pp4")
                nc.scalar.activation(pp4[:], ppNE[0:4, :], Copy)

                ptail = npsP.tile([128, 16], F32, tag="ptail")
                nc.tensor.transpose(ptail[:, 0:4], pp4[:, 0:128], ident4[:])
                nc.tensor.transpose(ptail[:, 4:8], pp4[:, 128:256], ident4[:])
                nt8 = nsb.tile([128, 8], F32, tag="nt8")
                nc.scalar.activation(nt8[:], ptail[:, 0:8], Copy)

                nc.tensor.matmul(ptail[:, 8:10], WgnT, nt8[:, 0:2], start=True, stop=False)
                nc.tensor.matmul(ptail[:, 8:10], WgeT, nt8[:, 6:8], start=False, stop=False)
                nc.tensor.matmul(ptail[:, 8:10], WggT, globT[:], start=False, stop=False)
                nc.tensor.matmul(ptail[:, 8:10], bgr, onesc, start=False, stop=True)
                outsb = nsb.tile([128, 2], F32, tag="outsb")
                nc.scalar.activation(outsb[:], ptail[:, 8:10], Copy)
                nc.sync.dma_start(d_out[:], outsb[:])

    return nc


_CACHE = {}


def _get_nc(NT, K0):
    key = (NT, K0)
    if key not in _CACHE:
        _CACHE[key] = _build(NT, K0)
    return _CACHE[key]


def _run(inputs, trace=False):
    in_maps, NT, K0, pairs = _prepare(inputs)
    nc = _get_nc(NT, K0)
    res = run_bass_kernel_spmd(nc, in_maps, list(range(N_CORES)), trace=trace)
    out = np.zeros((N_GRAPHS, 128), np.float32)
    for c in range(N_CORES):
        r = np.asarray(res.results[c]["out"], np.float32)
        pa, pb = pairs[c]
        out[pa] = r[:, 0]
        out[pb] = r[:, 1]
    return out, res


def kernel(**inputs):
    out, _ = _run(inputs, trace=False)
    return out


def kernel_traced(**inputs):
    return _run(inputs, trace=True)


# revision 41
# speedup vs baseline: 1.0785x; 1.0221x over previous
"""Trainium2 Bass kernel for a 2-layer GraphNetwork (gnn_message_passing).

Strategy (v2):
  - 16 graphs partitioned across 8 cores, 2 graphs per core, paired
    big+small by edge count to balance load. All segment reductions are
    core-local; [16,128] output rows are gathered on the host.
  - Per core, nodes are bin-packed (LPT) into NT tiles of 128 slots; each
    tile's incoming edges are padded to K0 chunks of 128. Segment sums run
    on the tensor engine as one-hot matmuls with HOST-built one-hot tiles.
  - Every hot-loop matmul uses a full K=128 stationary: edge/node feature
    tiles are zero-padded to 128 partitions, with ones/graph-indicator
    rows folded in so biases and global-feature terms are matmul
    accumulations against padded weight tiles. (Partial-K matmuls throttle
    the PE clock to 1.2 GHz; full-K keeps it at 2.4 GHz.)
  - bf16 inputs/intermediates, fp32 PSUM accumulation, fp32 final stage.
"""

import numpy as np
import ml_dtypes

import concourse.bass as bass
import concourse.tile as tile_mod
from concourse import tile
from concourse.bass_utils import run_bass_kernel_spmd
from concourse.vector_clock import ScopedClock

mybir = bass.mybir

N_NODES, N_EDGES, N_GRAPHS = 20000, 320000, 16
F_NODE, F_EDGE, F_GLOB = 64, 32, 16
N_CORES = 8
GPC = N_GRAPHS // N_CORES  # graphs per core = 2

BF16 = mybir.dt.bfloat16
FP8 = mybir.dt.float8e4
F32 = mybir.dt.float32
npbf16 = ml_dtypes.bfloat16
npfp8 = ml_dtypes.float8_e4m3

# ---------------------------------------------------------------------------
# Workaround: CoreV3 codegen rejects instructions carrying more than one
# semaphore wait. Split the waits across extra no-ops.
_MAX_WAITS = 1
_ENGINE_WAIT_LIMIT = 1
_SPLIT_ENGINES = None  # set lazily


def _split_excess_waits(nc):
    global _SPLIT_ENGINES
    if _SPLIT_ENGINES is None:
        ET = mybir.EngineType
        _SPLIT_ENGINES = {ET.PE, ET.Activation, ET.DVE, ET.SP, ET.Pool}
    ctr = [0]
    for bass_bb in nc.bb_map.values():
        bb = bass_bb.bb
        il = bb.instructions
        out = []
        changed = False
        for inst in il:
            si = inst.sync_info
            waits = list(si.on_wait) if (si and si.on_wait) else []
            if len(waits) > _ENGINE_WAIT_LIMIT and inst.engine in _SPLIT_ENGINES:
                head, keep = waits[:-_ENGINE_WAIT_LIMIT], waits[-_ENGINE_WAIT_LIMIT:]
                for i in range(0, len(head), _ENGINE_WAIT_LIMIT):
                    nop = mybir.InstNoOp(name=f"waitsplit-{ctr[0]}", ins=[], outs=[])
                    ctr[0] += 1
                    nop.engine = inst.engine
                    nop.sync_info = mybir.SyncInfo(
                        on_wait=head[i : i + _ENGINE_WAIT_LIMIT], on_update=[]
                    )
                    nc.register_instruction(nop, overwrite=True)
                    out.append(nop)
                inst.sync_info = mybir.SyncInfo(
                    on_wait=keep, on_update=list(si.on_update or [])
                )
                changed = True
            out.append(inst)
        if changed:
            bb.instructions = out


def _split_drain_and_barrier(self, tick_clock, wait_clock):
    nc = self.nc
    _split_excess_waits(nc)
    drain_inst = nc.sync.drain()
    wait_clock.add_sem_waits(
        drain_inst.ins, ScopedClock({None: tick_clock.global_clock})
    )
    mi = drain_inst.ins
    waits = list(mi.sync_info.on_wait) if (mi.sync_info and mi.sync_info.on_wait) else []
    if len(waits) > _MAX_WAITS:
        upd = list(mi.sync_info.on_update) if mi.sync_info.on_update else []
        mi.sync_info = mybir.SyncInfo(on_wait=waits[:_MAX_WAITS], on_update=upd)
        for i in range(_MAX_WAITS, len(waits), _MAX_WAITS):
            nop = nc.sync.nop(nofuse=True)
            nop.ins.sync_info = mybir.SyncInfo(
                on_wait=waits[i : i + _MAX_WAITS], on_update=[]
            )
    nc.all_engine_barrier()
    assert self.sems is not None
    popped = nc._tile_sem_poison_stack.pop()
    assert popped is self._sem_poison
    nc.clear_and_free_semaphores(list(self.sems.allocated().values()))
    nc.all_engine_barrier()


tile_mod.TileContext._drain_and_barrier = _split_drain_and_barrier


# ---------------------------------------------------------------------------
# Host-side graph partitioning / layout


def _pack_core(node_ids, degs, nt, cap_e):
    """LPT: place nodes (descending degree) onto the least-edge-loaded tile
    that still has node capacity. Returns per-tile node-id arrays, or None
    if some tile exceeds cap_e edges."""
    order = np.argsort(-degs, kind="stable")
    tiles_n = [[] for _ in range(nt)]
    tile_ncnt = np.zeros(nt, np.int64)
    tile_ecnt = np.zeros(nt, np.int64)
    for j in order:
        cand = np.where(tile_ncnt < 128)[0]
        if len(cand) == 0:
            return None
        t = cand[np.argmin(tile_ecnt[cand])]
        tiles_n[t].append(node_ids[j])
        tile_ncnt[t] += 1
        tile_ecnt[t] += degs[j]
    if (tile_ecnt > cap_e).any():
        return None
    return [np.array(t, dtype=np.int64) for t in tiles_n]


def _prepare(inputs):
    nf = np.asarray(inputs["node_feats"], np.float32)
    ef = np.asarray(inputs["edge_feats"], np.float32)
    glob = np.asarray(inputs["globals_"], np.float32)
    recv = np.asarray(inputs["receivers"]).astype(np.int64)
    ngraph = np.asarray(inputs["node_graph"]).astype(np.int64)

    cnt = np.bincount(recv, minlength=N_NODES).astype(np.int64)
    egraph = ngraph[recv]
    ncnt_g = np.bincount(ngraph, minlength=N_GRAPHS)
    ecnt_g = np.bincount(egraph, minlength=N_GRAPHS)

    # pair graphs big+small by edge count to balance cores
    order = np.argsort(-ecnt_g, kind="stable")
    pairs = [(int(order[i]), int(order[N_GRAPHS - 1 - i])) for i in range(N_CORES)]

    core_nodes = [
        np.where((ngraph == pa) | (ngraph == pb))[0] for pa, pb in pairs
    ]
    NT = int(max((len(cn) + 127) // 128 for cn in core_nodes))

    core_of_graph = np.zeros(N_GRAPHS, np.int64)
    for c, (pa, pb) in enumerate(pairs):
        core_of_graph[pa] = c
        core_of_graph[pb] = c
    edge_core = core_of_graph[egraph]
    ecnt_core = np.bincount(edge_core, minlength=N_CORES)

    packs = None
    K0 = max(1, int(max(ecnt_core) + NT * 128 - 1) // (NT * 128))
    K0 = ((K0 + 3) // 4) * 4  # quad-chunk device loops need K0 % 4 == 0
    for k0 in range(K0, K0 + 12, 4):
        trial = []
        ok = True
        for c in range(N_CORES):
            p = _pack_core(core_nodes[c], cnt[core_nodes[c]], NT, k0 * 128)
            if p is None:
                ok = False
                break
            trial.append(p)
        if ok:
            packs, K0 = trial, k0
            break
    assert packs is not None, "bin packing failed"

    NPAD = NT * 128
    EPAD = NT * K0 * 128

    # replicated weights, packed: bf16 block and f32 block
    w_np = {}

    def bf(x):
        return np.ascontiguousarray(x).astype(npbf16)

    We1T = np.zeros((128, 256), np.float32)
    We1T[:32] = np.asarray(inputs["We1"], np.float32).T
    We1T[32] = np.asarray(inputs["be1"], np.float32)

    We2T_ = np.asarray(inputs["We2"], np.float32).T  # [256, 128]
    We2T = np.concatenate([We2T_[:128], We2T_[128:]], axis=1)  # [128,256]

    Wn1T = np.zeros((128, 256), np.float32)
    Wn1T[:64] = np.asarray(inputs["Wn1"], np.float32).T
    Wn1T[64] = np.asarray(inputs["bn1"], np.float32)

    Win1T_ = np.asarray(inputs["Win1"], np.float32).T  # [256 fi, 256 fo]
    Win1T = np.concatenate(
        [Win1T_[:128, :128], Win1T_[:128, 128:], Win1T_[128:, :128], Win1T_[128:, 128:]],
        axis=1,
    )  # [128, 512]

    Wn2T_ = np.asarray(inputs["Wn2"], np.float32).T  # [256, 128]
    Wn2T = np.concatenate([Wn2T_[:128], Wn2T_[128:]], axis=1)  # [128,256]
    Win2T = np.asarray(inputs["Win2"], np.float32).T  # [128, 128]
    identb = np.eye(128, dtype=np.float32)

    WggT_pad = np.zeros((128, 128), np.float32)
    WggT_pad[:16] = np.asarray(inputs["Wgg"], np.float32).T
    bgr_pad = np.zeros((128, 128), np.float32)
    bgr_pad[0] = np.asarray(inputs["bg"], np.float32)
    onesc = np.zeros((128, 2), np.float32)
    onesc[0] = 1.0
    w_np["wf"] = np.ascontiguousarray(np.concatenate([
        np.asarray(inputs["Wgn"], np.float32).T,
        np.asarray(inputs["Wge"], np.float32).T,
        WggT_pad, bgr_pad, onesc,
    ], axis=1))  # [128, 514]
    w_np["ident4"] = np.eye(4, dtype=np.float32)

    Wg2 = np.asarray(inputs["Wg2"], np.float32)  # [128, 16]
    Wng2 = np.asarray(inputs["Wng2"], np.float32)
    be2 = np.asarray(inputs["be2"], np.float32)
    bn2 = np.asarray(inputs["bn2"], np.float32)

    # per-core inputs
    in_maps = []
    slot_of_node = np.full(N_NODES, -1, np.int64)
    tile_of_node = np.full(N_NODES, -1, np.int64)
    for c in range(N_CORES):
        pa, pb = pairs[c]
        for t in range(NT):
            ids = packs[c][t]
            slot_of_node[ids] = t * 128 + np.arange(len(ids))
            tile_of_node[ids] = t

        # ---- edges
        eidx = np.where(edge_core == c)[0]
        et = tile_of_node[recv[eidx]]
        order_e = np.argsort(et, kind="stable")
        eidx = eidx[order_e]
        et = et[order_e]
        counts = np.bincount(et, minlength=NT)
        starts = np.concatenate([[0], np.cumsum(counts)[:-1]])
        off_in = np.arange(len(eidx)) - np.repeat(starts, counts)
        dst = et * (K0 * 128) + off_in
        assert (counts <= K0 * 128).all()

        eftT = np.zeros((128, EPAD), np.float32)
        eftT[:32, dst] = ef[eidx].T
        eftT[32, dst] = 1.0
        eg = egraph[eidx]
        eftT[33, dst] = (eg == pa)
        eftT[34, dst] = (eg == pb)

        # one-hot: per 128-edge chunk a [lane, slot] block, K(partitions)=lanes
        onehot = np.zeros((128, EPAD), np.float32)
        onehot[dst % 128, (dst // 128) * 128 + slot_of_node[recv[eidx]] % 128] = 1.0

        # ---- nodes
        slot_node = np.full(NPAD, -1, np.int64)
        for t in range(NT):
            ids = packs[c][t]
            slot_node[t * 128 : t * 128 + len(ids)] = ids
        valid = slot_node >= 0
        sn = np.where(valid, slot_node, 0)

        nftT = np.zeros((128, NPAD), np.float32)
        nftT[:64, valid] = nf[sn[valid]].T
        nftT[64, valid] = 1.0
        ng = ngraph[sn]
        nftT[65] = valid * (ng == pa)
        nftT[66] = valid * (ng == pb)

        invc = np.ones((NPAD, 1), np.float32)
        invc[valid, 0] = 1.0 / np.maximum(cnt[sn[valid]], 1)

        poolw = np.zeros((NPAD, 128), np.float32)
        for g, gid in enumerate((pa, pb)):
            m = valid & (ng == gid)
            poolw[m, g] = 1.0 / max(ncnt_g[gid], 1)
            poolw[m, 2 + g] = cnt[sn[m]] / max(ecnt_g[gid], 1)

        g2aug = np.zeros((128, 128), np.float32)
        g2aug[32] = be2
        g2aug[33] = Wg2 @ glob[pa]
        g2aug[34] = Wg2 @ glob[pb]

        gnaug = np.zeros((128, 128), np.float32)
        gnaug[64] = bn2
        gnaug[65] = Wng2 @ glob[pa]
        gnaug[66] = Wng2 @ glob[pb]

        globT = np.zeros((128, 2), np.float32)
        globT[:16, 0] = glob[pa]
        globT[:16, 1] = glob[pb]

        wb = np.concatenate([We1T, g2aug, We2T, Wn1T, Win1T, Wn2T, Win2T,
                             gnaug, identb], axis=1)  # [128, 2048]

        m = {
            "eft": bf(eftT),
            "onehot": np.ascontiguousarray(onehot).astype(npfp8),
            "nft": bf(nftT),
            "invc": invc,
            "poolw": bf(poolw),
            "wb": bf(wb),
            "globT": globT,
        }
        m.update(w_np)
        in_maps.append(m)

    return in_maps, NT, K0, pairs


# ---------------------------------------------------------------------------
# Device program (identical on all cores)


def _build(NT, K0):
    nc = bass.Bass()
    NPAD = NT * 128
    EPAD = NT * K0 * 128
    CW = K0 * 128  # edge columns per node-tile

    d_eft = nc.dram_tensor("eft", [128, EPAD], BF16, kind="ExternalInput")
    d_onehot = nc.dram_tensor("onehot", [128, EPAD], FP8, kind="ExternalInput")
    d_nft = nc.dram_tensor("nft", [128, NPAD], BF16, kind="ExternalInput")
    d_invc = nc.dram_tensor("invc", [NPAD, 1], F32, kind="ExternalInput")
    d_poolw = nc.dram_tensor("poolw", [NPAD, 128], BF16, kind="ExternalInput")
    d_globT = nc.dram_tensor("globT", [128, 2], F32, kind="ExternalInput")

    d_wb = nc.dram_tensor("wb", [128, 2048], BF16, kind="ExternalInput")
    d_wf = nc.dram_tensor("wf", [128, 514], F32, kind="ExternalInput")
    d_ident4 = nc.dram_tensor("ident4", [4, 4], F32, kind="ExternalInput")

    d_out = nc.dram_tensor("out", [128, 2], F32, kind="ExternalOutput")

    Relu = mybir.ActivationFunctionType.Relu
    Copy = mybir.ActivationFunctionType.Copy
    NQ = K0 // 2   # e1 chunk-pairs per tile
    NR = K0 // 4   # e2 quads per tile
    NS = K0 // 2   # e1T 256-edge slabs per tile

    with tile.TileContext(nc) as tc:
        with tc.tile_pool(name="wp", bufs=1) as wp:
            wbt = wp.tile([128, 2048], BF16, tag="wb")
            nc.gpsimd.dma_start(wbt[:, 0:256], d_wb[:, 0:256])
            nc.gpsimd.dma_start(wbt[:, 256:2048], d_wb[:, 256:2048])
            wft = wp.tile([128, 514], F32, tag="wf")
            nc.gpsimd.dma_start(wft[:], d_wf[:])
            ident4 = wp.tile([4, 4], F32, tag="ident4")
            nc.gpsimd.dma_start(ident4[:], d_ident4[:])
            globT = wp.tile([128, 2], F32, tag="globT")
            nc.gpsimd.dma_start(globT[:], d_globT[:])
            We1T = wbt[:, 0:256]
            g2aug = wbt[:, 256:384]
            We1g = wbt[:, 0:384]
            We2T = wbt[:, 384:640]
            Wn1T = wbt[:, 640:896]
            Win1T = wbt[:, 896:1408]
            Wn2T = wbt[:, 1408:1664]
            Win2T = wbt[:, 1664:1792]
            gnaug = wbt[:, 1792:1920]
            identb = wbt[:, 1920:2048]
            WgnT = wft[:, 0:128]
            WgeT = wft[:, 128:256]
            WggT = wft[:, 256:384]
            bgr = wft[:, 384:512]
            onesc = wft[:, 512:514]

            aggall = wp.tile([128, 384 * NT], BF16, tag="aggall")

            # ----------------- edge phase -----------------
            with tc.tile_pool(name="ep", bufs=3) as ep, \
                 tc.tile_pool(name="e1p", bufs=2) as e1p, \
                 tc.tile_pool(name="efp", bufs=NQ + 2) as efp, \
                 tc.tile_pool(name="psA", bufs=2, space=bass.MemorySpace.PSUM) as psA, \
                 tc.tile_pool(name="psB", bufs=2, space=bass.MemorySpace.PSUM) as psB, \
                 tc.tile_pool(name="psAgg", bufs=2, space=bass.MemorySpace.PSUM) as psAgg:
                for t in range(NT):
                    eftt = ep.tile([128, CW], BF16, tag="eftt", bufs=2)
                    if t == 0:
                        nc.sync.dma_start(eftt[:, 0:512], d_eft[:, 0:512])
                        nc.sync.dma_start(eftt[:, 512:CW], d_eft[:, 512:CW])
                    else:
                        nc.sync.dma_start(eftt[:], d_eft[:, t * CW : (t + 1) * CW])
                    oht = ep.tile([128, CW], FP8, tag="oht", bufs=2)
                    nc.sync.dma_start(oht[:], d_onehot[:, t * CW : (t + 1) * CW])
                    invc_t = ep.tile([128, 1], F32, tag="invc")
                    nc.gpsimd.dma_start(invc_t[:], d_invc[t * 128 : (t + 1) * 128, :])

                    # e1T: feat-major, slab-blocked [h0(256e) | h1(256e)] per slab
                    e1h = e1p.tile([128, 2 * CW], BF16, tag="e1h")
                    for s in range(NS):
                        sl = slice(s * 256, (s + 1) * 256)
                        pT = psA.tile([128, 512], F32, tag="pT")
                        nc.tensor.matmul(pT[:, 0:256], We1T[:, 0:128],
                                         eftt[:, sl], start=True, stop=True)
                        nc.tensor.matmul(pT[:, 256:512], We1T[:, 128:256],
                                         eftt[:, sl], start=True, stop=True)
                        dst = e1h[:, s * 512 : (s + 1) * 512]
                        if s % 2 == 0:
                            nc.scalar.activation(dst, pT[:], Relu)
                        else:
                            nc.vector.tensor_scalar_max(dst, pT[:], 0.0)

                    # fused per-chunk: e1-pre + bias via one N=384 matmul, e2
                    # accumulates onto the bias region; one contiguous relu.
                    efs = []
                    for _q in range(NQ):
                        ef = efp.tile([128, 768], FP8, tag="ef")
                        efs.append(ef)
                    for c in range(K0):
                        csl = slice(c * 128, (c + 1) * 128)
                        pEc = psB.tile([128, 384], F32, tag="pEc", bufs=4)
                        nc.tensor.matmul(pEc[:], eftt[:, csl], We1g,
                                         start=True, stop=False, skip_group_check=True)
                        h0 = (c // 2) * 512 + (c % 2) * 128
                        nc.tensor.matmul(pEc[:, 256:384], e1h[:, h0 : h0 + 128],
                                         We2T[:, 0:128], start=False, stop=False,
                                         skip_group_check=True)
                        nc.tensor.matmul(pEc[:, 256:384], e1h[:, h0 + 256 : h0 + 384],
                                         We2T[:, 128:256], start=False, stop=True,
                                         skip_group_check=True)
                        dst = efs[c // 2][:, (c % 2) * 384 : (c % 2) * 384 + 384]
                        if c % 2 == 0:
                            nc.scalar.activation(dst, pEc[:], Relu)
                        else:
                            nc.vector.tensor_scalar_max(dst, pEc[:], 0.0)

                    # aggregation: fp8 DoubleRow, 256 edges (2 chunks) per matmul
                    pagg = psAgg.tile([128, 384], F32, tag="pagg")
                    for q in range(NQ):
                        lhs3 = oht[:, q * 256 : (q + 1) * 256].rearrange(
                            "k (p m) -> k p m", p=2)
                        rhs3 = efs[q][:].rearrange("k (p n) -> k p n", p=2)
                        nc.tensor.matmul(pagg[:], lhs3, rhs3,
                                         start=(q == 0), stop=(q == NQ - 1),
                                         perf_mode=mybir.MatmulPerfMode.DoubleRow)

                    nc.scalar.activation(
                        aggall[:, t * 384 : (t + 1) * 384], pagg[:], Copy,
                        scale=invc_t[:],
                    )

            # ----------------- node phase -----------------
            with tc.tile_pool(name="np_", bufs=NT) as np_, \
                 tc.tile_pool(name="agp", bufs=NT) as agp, \
                 tc.tile_pool(name="nsb", bufs=3) as nsb, \
                 tc.tile_pool(name="npsT", bufs=2, space=bass.MemorySpace.PSUM) as npsT, \
                 tc.tile_pool(name="npsB", bufs=2, space=bass.MemorySpace.PSUM) as npsB, \
                 tc.tile_pool(name="npsC", bufs=2, space=bass.MemorySpace.PSUM) as npsC, \
                 tc.tile_pool(name="npsP", bufs=1, space=bass.MemorySpace.PSUM) as npsP:
                ppNE = npsP.tile([128, 256], F32, tag="ppNE")

                nftts, pws = [], []
                for t in range(NT):
                    nftt = np_.tile([128, 128], BF16, tag="nftt")
                    nc.gpsimd.dma_start(nftt[:], d_nft[:, t * 128 : (t + 1) * 128])
                    pw = np_.tile([128, 128], BF16, tag="pw")
                    nc.gpsimd.dma_start(pw[:], d_poolw[t * 128 : (t + 1) * 128, :])
                    nftts.append(nftt)
                    pws.append(pw)

                # pass 1: transpose agg tiles to feat-major
                aggTs = []
                for t in range(NT):
                    aggsl = aggall[:, t * 384 : (t + 1) * 384]
                    pT = npsT.tile([128, 384], BF16, tag="pT")
                    nc.tensor.transpose(pT[:, 0:128], aggsl[:, 0:128], identb)
                    nc.tensor.transpose(pT[:, 128:256], aggsl[:, 128:256], identb)
                    nc.tensor.transpose(pT[:, 256:384], aggsl[:, 256:384], identb)
                    aggT = agp.tile([128, 384], BF16, tag="aggT")
                    nc.vector.tensor_copy(aggT[:], pT[:])
                    aggTs.append(aggT)

                # pass 2: node MLPs + pooling (pn1 emitted one tile ahead)
                def emit_pn1(t):
                    nftt = nftts[t]
                    aggT = aggTs[t]
                    pn1 = npsB.tile([128, 256], F32, tag="pn1")
                    for s in (0, 1):
                        ssl = slice(s * 128, (s + 1) * 128)
                        nc.tensor.matmul(pn1[:, ssl], Wn1T[:, ssl], nftt[:],
                                         start=True, stop=False)
                        nc.tensor.matmul(pn1[:, ssl], Win1T[:, s * 128 : s * 128 + 128],
                                         aggT[:, 0:128], start=False, stop=False)
                        nc.tensor.matmul(pn1[:, ssl], Win1T[:, 256 + s * 128 : 256 + s * 128 + 128],
                                         aggT[:, 128:256], start=False, stop=True)
                    return pn1

                pn1_cur = emit_pn1(0)
                for t in range(NT):
                    aggsl = aggall[:, t * 384 : (t + 1) * 384]
                    aggT = aggTs[t]
                    nftt = nftts[t]
                    pw = pws[t]

                    n1T = nsb.tile([128, 256], BF16, tag="n1T")
                    nc.scalar.activation(n1T[:], pn1_cur[:], Relu)
                    if t + 1 < NT:
                        pn1_cur = emit_pn1(t + 1)

                    pn2 = npsC.tile([128, 128], F32, tag="pn2")
                    nc.tensor.matmul(pn2[:], n1T[:, 0:128], Wn2T[:, 0:128], start=True, stop=False)
                    nc.tensor.matmul(pn2[:], n1T[:, 128:256], Wn2T[:, 128:256], start=False, stop=False)
                    nc.tensor.matmul(pn2[:], aggT[:, 256:384], Win2T, start=False, stop=False)
                    nc.tensor.matmul(pn2[:], nftt[:], gnaug, start=False, stop=True)
                    n2 = nsb.tile([128, 128], BF16, tag="n2")
                    nc.vector.tensor_scalar_max(n2[:], pn2[:], 0.0)

                    nc.tensor.matmul(ppNE[:, 0:128], pw[:], n2[:],
                                     start=(t == 0), stop=(t == NT - 1))

                # edge-average pooling: separate sequential group (same bank)
                for t in range(NT):
                    nc.tensor.matmul(ppNE[:, 128:256], pws[t][:],
                                     aggall[:, t * 384 + 256 : (t + 1) * 384],
                                     start=(t == 0), stop=(t == NT - 1))

                # ----------------- final projection -----------------
                pp4 = nsb.tile([4, 256], F32, tag="pp4")
                nc.scalar.activation(pp4[:], ppNE[0:4, :], Copy)

                ptail = npsP.tile([128, 16], F32, tag="ptail")
                nc.tensor.transpose(ptail[:, 0:4], pp4[:, 0:128], ident4[:])
                nc.tensor.transpose(ptail[:, 4:8], pp4[:, 128:256], ident4[:])
                nt8 = nsb.tile([128, 8], F32, tag="nt8")
                nc.scalar.activation(nt8[:], ptail[:, 0:8], Copy)

                nc.tensor.matmul(ptail[:, 8:10], WgnT, nt8[:, 0:2], start=True, stop=False)
                nc.tensor.matmul(ptail[:, 8:10], WgeT, nt8[:, 6:8], start=False, stop=False)
                nc.tensor.matmul(ptail[:, 8:10], WggT, globT[:], start=False, stop=False)
                nc.tensor.matmul(ptail[:, 8:10], bgr, onesc, start=False, stop=True)
                outsb = nsb.tile([128, 2], F32, tag="outsb")
                nc.scalar.activation(outsb[:], ptail[:, 8:10], Copy)
                nc.sync.dma_start(d_out[:], outsb[:])

    return nc


_CACHE = {}


def _get_nc(NT, K0):
    key = (NT, K0)
    if key not in _CACHE:
        _CACHE[key] = _build(NT, K0)
    return _CACHE[key]


def _run(inputs, trace=False):
    in_maps, NT, K0, pairs = _prepare(inputs)
    nc = _get_nc(NT, K0)
    res = run_bass_kernel_spmd(nc, in_maps, list(range(N_CORES)), trace=trace)
    out = np.zeros((N_GRAPHS, 128), np.float32)
    for c in range(N_CORES):
        r = np.asarray(res.results[c]["out"], np.float32)
        pa, pb = pairs[c]
        out[pa] = r[:, 0]
        out[pb] = r[:, 1]
    return out, res


def kernel(**inputs):
    out, _ = _run(inputs, trace=False)
    return out


def kernel_traced(**inputs):
    return _run(inputs, trace=True)
